# revision 24
# baseline (speedup 1.0000x reference)
"""FBANK kernel for Trainium2 (8 NeuronCores, pure data-parallel over batch).

Pipeline per core (8 batch rows): DFT-as-matmul in bf16 with selective
2x2-piece split precision for low fft bins (1..64), squares on ACT/DVE,
mel projection folded with the re^2+im^2 add via duplicated banks,
log+normalizer, and a tiny ragged masked-mean fixup via K=1/M=1 matmuls.
"""
import os
import numpy as np
import ml_dtypes

import concourse.bass as bass
import concourse.bacc as bacc
import concourse.tile as tile
from concourse import mybir
from concourse.bass_utils import run_bass_kernel_spmd

BF16 = ml_dtypes.bfloat16
F16 = np.float16

SR, WIN, SHIFT, NMEL, PRE = 16000.0, 400, 160, 80, 0.97
EPS = float(np.finfo(np.float64).eps)
B, L = 64, 240000
F = 1 + (L - WIN) // SHIFT          # 1498
ROWS = 8                            # batch rows per core
NCORES = 8
L_PAD = 240768                      # 1504*160 + slack for the +128 transpose
G = 1504                            # padded window-count per row
B0 = 64                             # fft bins 1..B0 get full split precision
NBINS = 199                         # fft bins 1..199 (bins 0, 200 have zero mel weight)

# C column layout: [cos1-64 | sin1-64 | cos65-192 | sin65-192 |
#                   cos193-199 | pad25 | sin193-199]  (sin block 32-aligned
#                   within M3 so the power-add partition bases are legal)
M_CHUNKS = 4       # M0 (precise, 128 cols), M1 (128), M2 (128), M3 (39)
M3_COLS = 39
NCOLS = 423
# k-chunks: (C row range, K, plane tile, g offset)
K_ROWS = [(0, 128), (160, 128), (320, 80), (128, 32), (288, 32)]
K_GOFF = [0, 1, 2, 0, 1]
K_TILE = ['lo', 'lo', 'lo', 'tl', 'tl']   # which X tile the rhs comes from

TILES = [(0, 512), (512, 512), (1024, 474)]   # (f0, NF) per row


def _build_consts():
    def hz2mel(f):
        return 1127.0 * np.log(1.0 + f / 700.0)
    mlow, mhigh = hz2mel(20.0), hz2mel(SR / 2.0)
    d = (mhigh - mlow) / (NMEL + 1)
    left = mlow + np.arange(NMEL) * d
    right = left + 2 * d
    fft_freqs = (SR / WIN) * np.arange(WIN // 2)
    mel = hz2mel(fft_freqs)[None, :]
    banks = np.maximum(0.0, np.minimum((mel - left[:, None]) / d,
                                       (right[:, None] - mel) / d))
    banks = np.concatenate([banks, np.zeros((NMEL, 1))], axis=1)  # (80, 201)

    w = 0.5 - 0.5 * np.cos(2 * np.pi * np.arange(WIN) / (WIN - 1))
    P = np.zeros((WIN, WIN))
    for t in range(WIN):
        P[t, t] += w[t]
        if t + 1 < WIN:
            P[t, t + 1] += -PRE * w[t + 1]
    P[0, 0] = (1 - PRE) * w[0]
    t_ = np.arange(WIN)
    k_ = np.arange(WIN // 2 + 1)
    ang = -2 * np.pi * np.outer(t_, k_) / WIN
    Cc = P @ np.cos(ang)          # (400, 201)
    Cs = P @ np.sin(ang)

    # reordered column layout (400, 423)
    C = np.concatenate([Cc[:, 1:B0 + 1], Cs[:, 1:B0 + 1],
                        Cc[:, B0 + 1:193], Cs[:, B0 + 1:193],
                        Cc[:, 193:200], np.zeros((WIN, 25)),
                        Cs[:, 193:200]], axis=1)
    C_h = C.astype(BF16)
    C_l = (C - C_h.astype(np.float64)).astype(BF16)

    # k-chunk stationary tiles (5, 128, NCOLS); rows beyond K zero
    ch = np.zeros((5, 128, NCOLS), BF16)
    cl = np.zeros((5, 128, 128), BF16)
    for i, (t0, K) in enumerate(K_ROWS):
        ch[i, :K, :] = C_h[t0:t0 + K, :]
        cl[i, :K, :] = C_l[t0:t0 + K, :128]

    bT = banks.T.astype(F16)      # (201, 80)
    # mel chunks over the squared DFT cols (duplicated banks fold re^2+im^2):
    # bd0 = [bins1-64 | bins1-64], bd1 = bins 65-192, bd2 = M3 39-row layout
    bd = np.zeros((3, 128, NMEL), F16)
    bd[0, 0:64] = bT[1:65];  bd[0, 64:128] = bT[1:65]
    bd[1, :] = bT[65:193]
    bd[2, 0:7] = bT[193:200]; bd[2, 32:39] = bT[193:200]
    return ch, cl, bd


_CH, _CL, _BD = _build_consts()
_NC = None


def _build_program():
    # Bacc (not Bass): its finalize() runs the sync-legalization passes
    # (move_matmul_waits_to_ldweights, generate_event_semaphores) that split
    # multi-wait instructions neuronxcc codegen rejects.
    nc = bacc.Bacc("TRN2", target_bir_lowering=False, debug=False)
    dt = mybir.dt
    xhlo = nc.dram_tensor("xhlo", [ROWS, 128, G], dt.bfloat16, kind="ExternalInput")
    xhtl = nc.dram_tensor("xhtl", [ROWS, 32, G], dt.bfloat16, kind="ExternalInput")
    xllo = nc.dram_tensor("xllo", [ROWS, 128, G], dt.bfloat16, kind="ExternalInput")
    xltl = nc.dram_tensor("xltl", [ROWS, 32, G], dt.bfloat16, kind="ExternalInput")
    ch = nc.dram_tensor("ch", [5, 128, NCOLS], dt.bfloat16, kind="ExternalInput")
    cl = nc.dram_tensor("cl", [5, 128, 128], dt.bfloat16, kind="ExternalInput")
    bd = nc.dram_tensor("bd", [3, 128, NMEL], dt.float16, kind="ExternalInput")
    nb = nc.dram_tensor("nb", [128, 12, NMEL], dt.float32, kind="ExternalInput")
    md = nc.dram_tensor("md", [128, ROWS], dt.float32, kind="ExternalInput")
    mb = nc.dram_tensor("mb", [1, ROWS * 128], dt.float32, kind="ExternalInput")
    out = nc.dram_tensor("out", [ROWS, F, NMEL], dt.float32, kind="ExternalOutput")

    from contextlib import ExitStack
    with tile.TileContext(nc) as tc, ExitStack() as ctx:
        singles = ctx.enter_context(tc.tile_pool(name="singles", bufs=1))
        xpool = ctx.enter_context(tc.tile_pool(name="xpool", bufs=8))
        sqpool = ctx.enter_context(tc.tile_pool(name="sqpool", bufs=4))
        fpool = ctx.enter_context(tc.tile_pool(name="fpool", bufs=3))
        dftps = ctx.enter_context(tc.tile_pool(name="dftps", bufs=1, space="PSUM"))
        melps = ctx.enter_context(tc.tile_pool(name="melps", bufs=2, space="PSUM"))
        fixps = ctx.enter_context(tc.tile_pool(name="fixps", bufs=1, space="PSUM"))

        ch_t = []
        cl_t = []
        for i in range(5):
            t = singles.tile([128, NCOLS], dt.bfloat16, tag=f"ch{i}")
            nc.sync.dma_start(t[:], ch[i])
            ch_t.append(t)
            t2 = singles.tile([128, 128], dt.bfloat16, tag=f"cl{i}")
            nc.sync.dma_start(t2[:], cl[i])
            cl_t.append(t2)
        bd_t = []
        for i in range(3):
            t = singles.tile([128, NMEL], dt.float16, tag=f"bd{i}")
            nc.sync.dma_start(t[:], bd[i])
            bd_t.append(t)
        nb_t = singles.tile([128, 12, NMEL], dt.float32, tag="nb")
        nc.sync.dma_start(nb_t[:], nb[:])
        md_t = singles.tile([128, ROWS], dt.float32, tag="md")
        nc.sync.dma_start(md_t[:], md[:])
        mb_t = singles.tile([1, ROWS * 128], dt.float32, tag="mb")
        nc.sync.dma_start(mb_t[:], mb[:])
        zeros_t = singles.tile([128, 1], dt.float32, tag="z")
        nc.vector.memset(zeros_t[:], 0.0)

        for r in range(ROWS):
            # --- load sample-major X planes (host pre-transposed); tl planes
            # only carry the 32 rows the K-chunks actually read ---
            xt = {}
            for pname, xsrc_lo, xsrc_tl in (("h", xhlo, xhtl), ("l", xllo, xltl)):
                xlo = xpool.tile([128, G], dt.bfloat16, tag=f"xlo{pname}")
                xtl = xpool.tile([32, G], dt.bfloat16, tag=f"xtl{pname}")
                nc.sync.dma_start(out=xlo[:], in_=xsrc_lo[r])
                nc.sync.dma_start(out=xtl[:], in_=xsrc_tl[r])
                xt[pname] = (xlo, xtl)

            stage = fpool.tile([128, 12, NMEL], dt.float32, tag="stage")
            nc.vector.memset(stage[64:128, 11, :], 1.0)
            for (f0, NF) in TILES:
                pA = dftps.tile([128, 2, 512], dt.float32, tag="pA")
                pB = dftps.tile([128, 2, 512], dt.float32, tag="pB")
                regions = [pA[:, 0, 0:NF], pA[:, 1, 0:NF],
                           pB[:, 0, 0:NF], pB[0:M3_COLS, 1, 0:NF]]
                mcols = [(0, 128), (128, 128), (256, 128), (384, M3_COLS)]

                def rhs_ap(plane, ki):
                    xlo, xtl = xt[plane]
                    t = xlo if K_TILE[ki] == 'lo' else xtl
                    K = K_ROWS[ki][1]
                    g = f0 + K_GOFF[ki]
                    return t[0:K, g:g + NF]

                # M0: 15 accumulating MMs (main + 2 cross passes)
                m0lo, m0n = mcols[0]
                for idx in range(15):
                    p, ki = idx // 5, idx % 5
                    K = K_ROWS[ki][1]
                    if p == 0:
                        lhs = ch_t[ki][0:K, m0lo:m0lo + m0n]
                        rhs = rhs_ap('h', ki)
                    elif p == 1:
                        lhs = cl_t[ki][0:K, 0:m0n]
                        rhs = rhs_ap('h', ki)
                    else:
                        lhs = ch_t[ki][0:K, m0lo:m0lo + m0n]
                        rhs = rhs_ap('l', ki)
                    nc.tensor.matmul(regions[0], lhs, rhs,
                                     start=(idx == 0), stop=(idx == 14))
                # M1..M3: 5 MMs each
                for mi in range(1, 4):
                    lo, n = mcols[mi]
                    for ki in range(5):
                        K = K_ROWS[ki][1]
                        nc.tensor.matmul(regions[mi],
                                         ch_t[ki][0:K, lo:lo + n],
                                         rhs_ap('h', ki),
                                         start=(ki == 0), stop=(ki == 4))

                # --- squares (psum fp32 -> sbuf fp16) on DVE; ACT was the
                # bottleneck engine for these wide elementwise ops ---
                sqA = sqpool.tile([128, 2, 512], dt.float16, tag="sqA")
                sqB = sqpool.tile([128, 2, 512], dt.float16, tag="sqB")
                cB = sqpool.tile([128, 2, 512], dt.float16, tag="cB")
                # Squares: DVE cannot read two PSUM operands, so the choice is
                # ACT Square (unary) or DVE copy-to-SBUF + mul. Alternate the
                # big regions between the two engines per tile to balance load.
                srcs = [(sqA, pA, 0, 128), (sqA, pA, 1, 128),
                        (sqB, pB, 0, 128), (sqB, pB, 1, M3_COLS)]
                on_act = (f0 // 512) % 2 == 0
                for si, (sq_t, p_t, pl, nr) in enumerate(srcs):
                    if on_act != (si % 2 == 0):
                        nc.scalar.activation(sq_t[0:nr, pl, 0:NF], p_t[0:nr, pl, 0:NF],
                                             mybir.ActivationFunctionType.Square,
                                             bias=zeros_t[0:nr])
                    else:
                        nc.vector.tensor_copy(cB[0:nr, pl, 0:NF], p_t[0:nr, pl, 0:NF])
                        nc.vector.tensor_mul(sq_t[0:nr, pl, 0:NF], cB[0:nr, pl, 0:NF],
                                             cB[0:nr, pl, 0:NF])

                # --- mel matmuls: out[frame, mel], 4 K-chunks over squares ---
                mps = melps.tile([128, 4, NMEL], dt.float32, tag="mps")
                nsub = (NF + 127) // 128
                for j in range(nsub):
                    nj = min(128, NF - j * 128)
                    fr = slice(j * 128, j * 128 + nj)
                    chunks = [(sqA[:, 0, fr], 128, bd_t[0]),
                              (sqA[:, 1, fr], 128, bd_t[1]),
                              (sqB[:, 0, fr], 128, bd_t[1]),
                              (sqB[0:M3_COLS, 1, fr], M3_COLS, bd_t[2])]
                    for ci, (lhs, K, bdt) in enumerate(chunks):
                        nc.tensor.matmul(mps[0:nj, j, :], lhs, bdt[0:K, :],
                                         start=(ci == 0), stop=(ci == 3))

                # --- clamp into per-row staging (Ln/mul happen once per row) ---
                ti = f0 // 512
                for j in range(nsub):
                    nj = min(128, NF - j * 128)
                    nc.vector.tensor_scalar_max(stage[0:nj, ti * 4 + j, :],
                                                mps[0:nj, j, :], EPS)

            # --- row tail: one Ln + one normalizer mul over all 12 subtiles ---
            out_st = fpool.tile([128, 12, NMEL], dt.float32, tag="out_st")
            nc.scalar.activation(out_st[:], stage[:],
                                 mybir.ActivationFunctionType.Ln,
                                 bias=zeros_t[:])
            nc.vector.tensor_mul(out_st[:], out_st[:], nb_t[:])

            # --- ragged masked-mean fixup (frames 0..127 = subtile 0) ---
            mean_ps = fixps.tile([1, NMEL], dt.float32, tag="mean")
            nc.tensor.matmul(mean_ps[:], md_t[:, r:r + 1], out_st[:, 0, :],
                             start=True, stop=True)
            mean_sb = fpool.tile([1, NMEL], dt.float32, tag="mean_sb")
            nc.vector.tensor_copy(mean_sb[:], mean_ps[:])
            fix_ps = fixps.tile([128, NMEL], dt.float32, tag="fix")
            nc.tensor.matmul(fix_ps[:], mb_t[0:1, r * 128:(r + 1) * 128],
                             mean_sb[:], start=True, stop=True)
            nc.vector.tensor_sub(out_st[:, 0, :], out_st[:, 0, :], fix_ps[:])

            # --- stores ---
            for js in range(12):
                nj = min(128, F - js * 128)
                nc.sync.dma_start(out[r, js * 128: js * 128 + nj, :],
                                  out_st[0:nj, js, :])
    nc.finalize()
    return nc


def _sample_major(plane):
    # plane (B, L_PAD) -> lo (B, 128, G): [b, v, g] = plane[b, 160g + v]
    #                     tl (B, 32, G):  [b, v, g] = plane[b, 160g + 128 + v]
    s = plane.strides
    lo = np.lib.stride_tricks.as_strided(
        plane, shape=(B, 128, G), strides=(s[0], s[1], 160 * s[1]))
    tl = np.lib.stride_tricks.as_strided(
        plane[:, 128:], shape=(B, 32, G), strides=(s[0], s[1], 160 * s[1]))
    return np.ascontiguousarray(lo), np.ascontiguousarray(tl)


def _host_prep(x, T, normalizer):
    xf = np.asarray(x, np.float32)
    xh0 = np.zeros((B, L_PAD), BF16)
    xh0[:, :L] = xf.astype(BF16)
    xl0 = np.zeros((B, L_PAD), BF16)
    xl0[:, :L] = (xf - xh0[:, :L].astype(np.float32)).astype(BF16)
    xh = _sample_major(xh0)
    xlo = _sample_major(xl0)

    T = np.asarray(T, np.int32)
    ds = T.max().astype(np.float32) / np.float32(NMEL)
    T_ = (T.astype(np.float32) / ds).astype(np.int32)
    cnt = np.maximum(T_, 1).astype(np.float32)
    f = np.arange(128)[None, :]
    maskbit = (f < T_[:, None]).astype(np.float32)          # (64, 128)
    maskdiv = maskbit / cnt[:, None]

    nrm = np.asarray(normalizer, np.float32)
    nb = np.broadcast_to(nrm[None, None, :], (128, 12, NMEL)).copy()
    return xh, xlo, maskdiv, maskbit, nb


_PMAP = None


def _jax_kernel(x, T, normalizer):
    """Data-parallel FBANK on the 8 NeuronCores via jax pmap (conv-based
    framing+DFT; no gather)."""
    global _PMAP
    import jax
    import jax.numpy as jnp

    def hz2mel(f):
        return 1127.0 * np.log(1.0 + f / 700.0)
    mlow, mhigh = hz2mel(20.0), hz2mel(SR / 2.0)
    d = (mhigh - mlow) / (NMEL + 1)
    left = mlow + np.arange(NMEL) * d
    fft_freqs = (SR / WIN) * np.arange(WIN // 2)
    melf = hz2mel(fft_freqs)[None, :]
    banks = np.maximum(0.0, np.minimum((melf - left[:, None]) / d,
                                       ((left[:, None] + 2 * d) - melf) / d))
    banksT = banks.T[1:200].astype(np.float32)           # (199, 80)

    w = 0.5 - 0.5 * np.cos(2 * np.pi * np.arange(WIN) / (WIN - 1))
    P = np.zeros((WIN, WIN))
    for t in range(WIN):
        P[t, t] += w[t]
        if t + 1 < WIN:
            P[t, t + 1] += -PRE * w[t + 1]
    P[0, 0] = (1 - PRE) * w[0]
    ang = -2 * np.pi * np.outer(np.arange(WIN), np.arange(1, 200)) / WIN
    Cc = (P @ np.cos(ang)).astype(np.float32)            # (400, 199)
    Cs = (P @ np.sin(ang)).astype(np.float32)
    filt = np.concatenate([Cc, Cs], axis=1).T[:, None, :]  # (398, 1, 400)

    if _PMAP is None:
        filt_j = jnp.asarray(filt)
        banksT_j = jnp.asarray(banksT)

        def shard_fn(xs, mdiv, mbit, nrm):
            spec = jax.lax.conv_general_dilated(
                xs[:, None, :], filt_j, window_strides=(SHIFT,),
                padding="VALID")                          # (r, 398, F)
            re, im = spec[:, :199, :], spec[:, 199:, :]
            power = re * re + im * im                     # (r, 199, F)
            mel = jnp.einsum("bkf,km->bfm", power, banksT_j)
            fb = jnp.log(jnp.maximum(mel, EPS)) * nrm[:, None, :]
            mean = jnp.einsum("bfm,bf->bm", fb, mdiv)[:, None, :]
            return fb - mean * mbit[:, :, None]

        _PMAP = jax.pmap(shard_fn)

    xf = np.asarray(x, np.float32).reshape(NCORES, ROWS, L)
    T = np.asarray(T, np.int32)
    ds = T.max().astype(np.float32) / np.float32(NMEL)
    T_ = (T.astype(np.float32) / ds).astype(np.int32)
    cnt = np.maximum(T_, 1).astype(np.float32)
    f = np.arange(F)[None, :]
    mbit = (f < T_[:, None]).astype(np.float32)           # (64, F)
    mdiv = mbit / cnt[:, None]
    nrm = np.broadcast_to(np.asarray(normalizer, np.float32)[None, :], (B, NMEL))
    out = _PMAP(xf, mdiv.reshape(NCORES, ROWS, F), mbit.reshape(NCORES, ROWS, F),
                nrm.reshape(NCORES, ROWS, NMEL))
    return np.asarray(out).reshape(B, F, NMEL).astype(np.float32)


def _in_maps(x, T, normalizer):
    (xh_lo, xh_tl), (xl_lo, xl_tl), maskdiv, maskbit, nb = _host_prep(x, T, normalizer)
    in_maps = []
    for c in range(NCORES):
        r0 = c * ROWS
        sl = slice(r0, r0 + ROWS)
        in_maps.append({
            "xhlo": xh_lo[sl], "xhtl": xh_tl[sl],
            "xllo": xl_lo[sl], "xltl": xl_tl[sl],
            "ch": _CH, "cl": _CL, "bd": _BD, "nb": nb,
            "md": np.ascontiguousarray(maskdiv[sl].T),
            "mb": maskbit[sl].reshape(1, -1),
        })
    return in_maps


def kernel(x, T, normalizer):
    if int(os.environ.get("KERNEL_PMAP", "0")):
        return _jax_kernel(x, T, normalizer)
    global _NC
    if _NC is None:
        _NC = _build_program()
    in_maps = _in_maps(x, T, normalizer)
    trace = bool(int(os.environ.get("KERNEL_TRACE", "0")))
    res = run_bass_kernel_spmd(_NC, in_maps, list(range(NCORES)), trace=trace)
    if res.exec_time_ns is not None:
        print(f"HW exec time: {res.exec_time_ns} ns")
    return np.concatenate([res.results[c]["out"] for c in range(NCORES)], axis=0)



# revision 25
# speedup vs baseline: 1.0632x; 1.0632x over previous
"""FBANK kernel for Trainium2 (8 NeuronCores, pure data-parallel over batch).

Pipeline per core (8 batch rows): DFT-as-matmul in bf16 with selective
2x2-piece split precision for low fft bins (1..64), squares on ACT/DVE,
mel projection folded with the re^2+im^2 add via duplicated banks,
log+normalizer, and a tiny ragged masked-mean fixup via K=1/M=1 matmuls.
"""
import os
import numpy as np
import ml_dtypes

import concourse.bass as bass
import concourse.bacc as bacc
import concourse.tile as tile
from concourse import mybir
from concourse.bass_utils import run_bass_kernel_spmd

BF16 = ml_dtypes.bfloat16
F16 = np.float16

SR, WIN, SHIFT, NMEL, PRE = 16000.0, 400, 160, 80, 0.97
EPS = float(np.finfo(np.float64).eps)
B, L = 64, 240000
F = 1 + (L - WIN) // SHIFT          # 1498
ROWS = 8                            # batch rows per core
NCORES = 8
L_PAD = 240768                      # 1504*160 + slack for the +128 transpose
G = 1504                            # padded window-count per row
B0 = 64                             # fft bins 1..B0 get full split precision
NBINS = 199                         # fft bins 1..199 (bins 0, 200 have zero mel weight)

# C column layout: [cos1-64 | sin1-64 | cos65-192 | sin65-192 |
#                   cos193-199 | pad25 | sin193-199]  (sin block 32-aligned
#                   within M3 so the power-add partition bases are legal)
M_CHUNKS = 4       # M0 (precise, 128 cols), M1 (128), M2 (128), M3 (39)
M3_COLS = 39
NCOLS = 423
# k-chunks: (C row range, K, plane tile, g offset)
K_ROWS = [(0, 128), (160, 128), (320, 80), (128, 32), (288, 32)]
K_GOFF = [0, 1, 2, 0, 1]
K_TILE = ['lo', 'lo', 'lo', 'tl', 'tl']   # which X tile the rhs comes from

TILES = [(0, 512), (512, 512), (1024, 474)]   # (f0, NF) per row


def _build_consts():
    def hz2mel(f):
        return 1127.0 * np.log(1.0 + f / 700.0)
    mlow, mhigh = hz2mel(20.0), hz2mel(SR / 2.0)
    d = (mhigh - mlow) / (NMEL + 1)
    left = mlow + np.arange(NMEL) * d
    right = left + 2 * d
    fft_freqs = (SR / WIN) * np.arange(WIN // 2)
    mel = hz2mel(fft_freqs)[None, :]
    banks = np.maximum(0.0, np.minimum((mel - left[:, None]) / d,
                                       (right[:, None] - mel) / d))
    banks = np.concatenate([banks, np.zeros((NMEL, 1))], axis=1)  # (80, 201)

    w = 0.5 - 0.5 * np.cos(2 * np.pi * np.arange(WIN) / (WIN - 1))
    P = np.zeros((WIN, WIN))
    for t in range(WIN):
        P[t, t] += w[t]
        if t + 1 < WIN:
            P[t, t + 1] += -PRE * w[t + 1]
    P[0, 0] = (1 - PRE) * w[0]
    t_ = np.arange(WIN)
    k_ = np.arange(WIN // 2 + 1)
    ang = -2 * np.pi * np.outer(t_, k_) / WIN
    Cc = P @ np.cos(ang)          # (400, 201)
    Cs = P @ np.sin(ang)

    # reordered column layout (400, 423)
    C = np.concatenate([Cc[:, 1:B0 + 1], Cs[:, 1:B0 + 1],
                        Cc[:, B0 + 1:193], Cs[:, B0 + 1:193],
                        Cc[:, 193:200], np.zeros((WIN, 25)),
                        Cs[:, 193:200]], axis=1)
    C_h = C.astype(BF16)
    C_l = (C - C_h.astype(np.float64)).astype(BF16)

    # k-chunk stationary tiles (5, 128, NCOLS); rows beyond K zero
    ch = np.zeros((5, 128, NCOLS), BF16)
    cl = np.zeros((5, 128, 128), BF16)
    for i, (t0, K) in enumerate(K_ROWS):
        ch[i, :K, :] = C_h[t0:t0 + K, :]
        cl[i, :K, :] = C_l[t0:t0 + K, :128]

    bT = banks.T.astype(F16)      # (201, 80)
    # mel chunks over the squared DFT cols (duplicated banks fold re^2+im^2):
    # bd0 = [bins1-64 | bins1-64], bd1 = bins 65-192, bd2 = M3 39-row layout
    bd = np.zeros((3, 128, NMEL), F16)
    bd[0, 0:64] = bT[1:65];  bd[0, 64:128] = bT[1:65]
    bd[1, :] = bT[65:193]
    bd[2, 0:7] = bT[193:200]; bd[2, 32:39] = bT[193:200]
    return ch, cl, bd


_CH, _CL, _BD = _build_consts()
_NC = None


def _build_program():
    # Bacc (not Bass): its finalize() runs the sync-legalization passes
    # (move_matmul_waits_to_ldweights, generate_event_semaphores) that split
    # multi-wait instructions neuronxcc codegen rejects.
    nc = bacc.Bacc("TRN2", target_bir_lowering=False, debug=False)
    dt = mybir.dt
    xhlo = nc.dram_tensor("xhlo", [ROWS, 128, G], dt.bfloat16, kind="ExternalInput")
    xhtl = nc.dram_tensor("xhtl", [ROWS, 32, G], dt.bfloat16, kind="ExternalInput")
    xllo = nc.dram_tensor("xllo", [ROWS, 128, G], dt.bfloat16, kind="ExternalInput")
    xltl = nc.dram_tensor("xltl", [ROWS, 32, G], dt.bfloat16, kind="ExternalInput")
    ch = nc.dram_tensor("ch", [5, 128, NCOLS], dt.bfloat16, kind="ExternalInput")
    cl = nc.dram_tensor("cl", [5, 128, 128], dt.bfloat16, kind="ExternalInput")
    bd = nc.dram_tensor("bd", [3, 128, NMEL], dt.float16, kind="ExternalInput")
    nb = nc.dram_tensor("nb", [128, 12, NMEL], dt.float32, kind="ExternalInput")
    md = nc.dram_tensor("md", [128, ROWS], dt.float32, kind="ExternalInput")
    mb = nc.dram_tensor("mb", [1, ROWS * 128], dt.float32, kind="ExternalInput")
    out = nc.dram_tensor("out", [ROWS, F, NMEL], dt.float32, kind="ExternalOutput")

    from contextlib import ExitStack
    with tile.TileContext(nc) as tc, ExitStack() as ctx:
        singles = ctx.enter_context(tc.tile_pool(name="singles", bufs=1))
        xpool = ctx.enter_context(tc.tile_pool(name="xpool", bufs=8))
        sqpool = ctx.enter_context(tc.tile_pool(name="sqpool", bufs=6))
        fpool = ctx.enter_context(tc.tile_pool(name="fpool", bufs=3))
        dftps = ctx.enter_context(tc.tile_pool(name="dftps", bufs=1, space="PSUM"))
        melps = ctx.enter_context(tc.tile_pool(name="melps", bufs=2, space="PSUM"))
        fixps = ctx.enter_context(tc.tile_pool(name="fixps", bufs=1, space="PSUM"))

        ch_t = []
        cl_t = []
        for i in range(5):
            t = singles.tile([128, NCOLS], dt.bfloat16, tag=f"ch{i}")
            nc.sync.dma_start(t[:], ch[i])
            ch_t.append(t)
            t2 = singles.tile([128, 128], dt.bfloat16, tag=f"cl{i}")
            nc.sync.dma_start(t2[:], cl[i])
            cl_t.append(t2)
        bd_t = []
        for i in range(3):
            t = singles.tile([128, NMEL], dt.float16, tag=f"bd{i}")
            nc.sync.dma_start(t[:], bd[i])
            bd_t.append(t)
        nb_t = singles.tile([128, 12, NMEL], dt.float32, tag="nb")
        nc.sync.dma_start(nb_t[:], nb[:])
        md_t = singles.tile([128, ROWS], dt.float32, tag="md")
        nc.sync.dma_start(md_t[:], md[:])
        mb_t = singles.tile([1, ROWS * 128], dt.float32, tag="mb")
        nc.sync.dma_start(mb_t[:], mb[:])
        zeros_t = singles.tile([128, 1], dt.float32, tag="z")
        nc.vector.memset(zeros_t[:], 0.0)

        for r in range(ROWS):
            # --- load sample-major X planes (host pre-transposed); tl planes
            # only carry the 32 rows the K-chunks actually read ---
            xt = {}
            for pname, xsrc_lo, xsrc_tl in (("h", xhlo, xhtl), ("l", xllo, xltl)):
                xlo = xpool.tile([128, G], dt.bfloat16, tag=f"xlo{pname}")
                xtl = xpool.tile([32, G], dt.bfloat16, tag=f"xtl{pname}")
                nc.sync.dma_start(out=xlo[:], in_=xsrc_lo[r])
                nc.sync.dma_start(out=xtl[:], in_=xsrc_tl[r])
                xt[pname] = (xlo, xtl)

            stage = fpool.tile([128, 12, NMEL], dt.float32, tag="stage")
            nc.vector.memset(stage[64:128, 11, :], 1.0)
            for (f0, NF) in TILES:
                pA = dftps.tile([128, 2, 512], dt.float32, tag="pA")
                pB = dftps.tile([128, 2, 512], dt.float32, tag="pB")
                regions = [pA[:, 0, 0:NF], pA[:, 1, 0:NF],
                           pB[:, 0, 0:NF], pB[0:M3_COLS, 1, 0:NF]]
                mcols = [(0, 128), (128, 128), (256, 128), (384, M3_COLS)]

                def rhs_ap(plane, ki):
                    xlo, xtl = xt[plane]
                    t = xlo if K_TILE[ki] == 'lo' else xtl
                    K = K_ROWS[ki][1]
                    g = f0 + K_GOFF[ki]
                    return t[0:K, g:g + NF]

                # M0: 15 accumulating MMs (main + 2 cross passes)
                m0lo, m0n = mcols[0]
                for idx in range(15):
                    p, ki = idx // 5, idx % 5
                    K = K_ROWS[ki][1]
                    if p == 0:
                        lhs = ch_t[ki][0:K, m0lo:m0lo + m0n]
                        rhs = rhs_ap('h', ki)
                    elif p == 1:
                        lhs = cl_t[ki][0:K, 0:m0n]
                        rhs = rhs_ap('h', ki)
                    else:
                        lhs = ch_t[ki][0:K, m0lo:m0lo + m0n]
                        rhs = rhs_ap('l', ki)
                    nc.tensor.matmul(regions[0], lhs, rhs,
                                     start=(idx == 0), stop=(idx == 14))
                # M1..M3: 5 MMs each
                for mi in range(1, 4):
                    lo, n = mcols[mi]
                    for ki in range(5):
                        K = K_ROWS[ki][1]
                        nc.tensor.matmul(regions[mi],
                                         ch_t[ki][0:K, lo:lo + n],
                                         rhs_ap('h', ki),
                                         start=(ki == 0), stop=(ki == 4))

                # --- squares (psum fp32 -> sbuf fp16) on DVE; ACT was the
                # bottleneck engine for these wide elementwise ops ---
                sqA = sqpool.tile([128, 2, 512], dt.float16, tag="sqA")
                sqB = sqpool.tile([128, 2, 512], dt.float16, tag="sqB")
                cB = sqpool.tile([128, 2, 512], dt.float16, tag="cB")
                # Squares: DVE cannot read two PSUM operands, so the choice is
                # ACT Square (unary) or DVE copy-to-SBUF + mul. Alternate the
                # big regions between the two engines per tile to balance load.
                srcs = [(sqA, pA, 0, 128), (sqA, pA, 1, 128),
                        (sqB, pB, 0, 128), (sqB, pB, 1, M3_COLS)]
                on_act = (f0 // 512) % 2 == 0
                for si, (sq_t, p_t, pl, nr) in enumerate(srcs):
                    if on_act != (si % 2 == 0):
                        nc.scalar.activation(sq_t[0:nr, pl, 0:NF], p_t[0:nr, pl, 0:NF],
                                             mybir.ActivationFunctionType.Square,
                                             bias=zeros_t[0:nr])
                    else:
                        nc.vector.tensor_copy(cB[0:nr, pl, 0:NF], p_t[0:nr, pl, 0:NF])
                        nc.vector.tensor_mul(sq_t[0:nr, pl, 0:NF], cB[0:nr, pl, 0:NF],
                                             cB[0:nr, pl, 0:NF])

                # --- power pre-add for bins 65-192 (both operands base 0,
                # so the tensor_tensor SBUF base-partition rule is satisfied) ---
                pw = sqpool.tile([128, 512], dt.float16, tag="pw")
                nc.gpsimd.tensor_add(pw[:, 0:NF], sqA[:, 1, 0:NF], sqB[:, 0, 0:NF])

                # --- mel matmuls: out[frame, mel], 3 K-chunks ---
                mps = melps.tile([128, 4, NMEL], dt.float32, tag="mps")
                nsub = (NF + 127) // 128
                for j in range(nsub):
                    nj = min(128, NF - j * 128)
                    fr = slice(j * 128, j * 128 + nj)
                    chunks = [(sqA[:, 0, fr], 128, bd_t[0]),
                              (pw[:, fr], 128, bd_t[1]),
                              (sqB[0:M3_COLS, 1, fr], M3_COLS, bd_t[2])]
                    for ci, (lhs, K, bdt) in enumerate(chunks):
                        nc.tensor.matmul(mps[0:nj, j, :], lhs, bdt[0:K, :],
                                         start=(ci == 0), stop=(ci == 2))

                # --- clamp into per-row staging (Ln/mul happen once per row) ---
                ti = f0 // 512
                for j in range(nsub):
                    nj = min(128, NF - j * 128)
                    nc.vector.tensor_scalar_max(stage[0:nj, ti * 4 + j, :],
                                                mps[0:nj, j, :], EPS)

            # --- row tail: one Ln + one normalizer mul over all 12 subtiles ---
            out_st = fpool.tile([128, 12, NMEL], dt.float32, tag="out_st")
            nc.scalar.activation(out_st[:], stage[:],
                                 mybir.ActivationFunctionType.Ln,
                                 bias=zeros_t[:])
            nc.vector.tensor_mul(out_st[:], out_st[:], nb_t[:])

            # --- ragged masked-mean fixup (frames 0..127 = subtile 0) ---
            mean_ps = fixps.tile([1, NMEL], dt.float32, tag="mean")
            nc.tensor.matmul(mean_ps[:], md_t[:, r:r + 1], out_st[:, 0, :],
                             start=True, stop=True)
            mean_sb = fpool.tile([1, NMEL], dt.float32, tag="mean_sb")
            nc.vector.tensor_copy(mean_sb[:], mean_ps[:])
            fix_ps = fixps.tile([128, NMEL], dt.float32, tag="fix")
            nc.tensor.matmul(fix_ps[:], mb_t[0:1, r * 128:(r + 1) * 128],
                             mean_sb[:], start=True, stop=True)
            nc.vector.tensor_sub(out_st[:, 0, :], out_st[:, 0, :], fix_ps[:])

            # --- stores ---
            for js in range(12):
                nj = min(128, F - js * 128)
                nc.sync.dma_start(out[r, js * 128: js * 128 + nj, :],
                                  out_st[0:nj, js, :])
    nc.finalize()
    return nc


def _sample_major(plane):
    # plane (B, L_PAD) -> lo (B, 128, G): [b, v, g] = plane[b, 160g + v]
    #                     tl (B, 32, G):  [b, v, g] = plane[b, 160g + 128 + v]
    s = plane.strides
    lo = np.lib.stride_tricks.as_strided(
        plane, shape=(B, 128, G), strides=(s[0], s[1], 160 * s[1]))
    tl = np.lib.stride_tricks.as_strided(
        plane[:, 128:], shape=(B, 32, G), strides=(s[0], s[1], 160 * s[1]))
    return np.ascontiguousarray(lo), np.ascontiguousarray(tl)


def _host_prep(x, T, normalizer):
    xf = np.asarray(x, np.float32)
    xh0 = np.zeros((B, L_PAD), BF16)
    xh0[:, :L] = xf.astype(BF16)
    xl0 = np.zeros((B, L_PAD), BF16)
    xl0[:, :L] = (xf - xh0[:, :L].astype(np.float32)).astype(BF16)
    xh = _sample_major(xh0)
    xlo = _sample_major(xl0)

    T = np.asarray(T, np.int32)
    ds = T.max().astype(np.float32) / np.float32(NMEL)
    T_ = (T.astype(np.float32) / ds).astype(np.int32)
    cnt = np.maximum(T_, 1).astype(np.float32)
    f = np.arange(128)[None, :]
    maskbit = (f < T_[:, None]).astype(np.float32)          # (64, 128)
    maskdiv = maskbit / cnt[:, None]

    nrm = np.asarray(normalizer, np.float32)
    nb = np.broadcast_to(nrm[None, None, :], (128, 12, NMEL)).copy()
    return xh, xlo, maskdiv, maskbit, nb


_PMAP = None


def _jax_kernel(x, T, normalizer):
    """Data-parallel FBANK on the 8 NeuronCores via jax pmap (conv-based
    framing+DFT; no gather)."""
    global _PMAP
    import jax
    import jax.numpy as jnp

    def hz2mel(f):
        return 1127.0 * np.log(1.0 + f / 700.0)
    mlow, mhigh = hz2mel(20.0), hz2mel(SR / 2.0)
    d = (mhigh - mlow) / (NMEL + 1)
    left = mlow + np.arange(NMEL) * d
    fft_freqs = (SR / WIN) * np.arange(WIN // 2)
    melf = hz2mel(fft_freqs)[None, :]
    banks = np.maximum(0.0, np.minimum((melf - left[:, None]) / d,
                                       ((left[:, None] + 2 * d) - melf) / d))
    banksT = banks.T[1:200].astype(np.float32)           # (199, 80)

    w = 0.5 - 0.5 * np.cos(2 * np.pi * np.arange(WIN) / (WIN - 1))
    P = np.zeros((WIN, WIN))
    for t in range(WIN):
        P[t, t] += w[t]
        if t + 1 < WIN:
            P[t, t + 1] += -PRE * w[t + 1]
    P[0, 0] = (1 - PRE) * w[0]
    ang = -2 * np.pi * np.outer(np.arange(WIN), np.arange(1, 200)) / WIN
    Cc = (P @ np.cos(ang)).astype(np.float32)            # (400, 199)
    Cs = (P @ np.sin(ang)).astype(np.float32)
    filt = np.concatenate([Cc, Cs], axis=1).T[:, None, :]  # (398, 1, 400)

    if _PMAP is None:
        filt_j = jnp.asarray(filt)
        banksT_j = jnp.asarray(banksT)

        def shard_fn(xs, mdiv, mbit, nrm):
            spec = jax.lax.conv_general_dilated(
                xs[:, None, :], filt_j, window_strides=(SHIFT,),
                padding="VALID")                          # (r, 398, F)
            re, im = spec[:, :199, :], spec[:, 199:, :]
            power = re * re + im * im                     # (r, 199, F)
            mel = jnp.einsum("bkf,km->bfm", power, banksT_j)
            fb = jnp.log(jnp.maximum(mel, EPS)) * nrm[:, None, :]
            mean = jnp.einsum("bfm,bf->bm", fb, mdiv)[:, None, :]
            return fb - mean * mbit[:, :, None]

        _PMAP = jax.pmap(shard_fn)

    xf = np.asarray(x, np.float32).reshape(NCORES, ROWS, L)
    T = np.asarray(T, np.int32)
    ds = T.max().astype(np.float32) / np.float32(NMEL)
    T_ = (T.astype(np.float32) / ds).astype(np.int32)
    cnt = np.maximum(T_, 1).astype(np.float32)
    f = np.arange(F)[None, :]
    mbit = (f < T_[:, None]).astype(np.float32)           # (64, F)
    mdiv = mbit / cnt[:, None]
    nrm = np.broadcast_to(np.asarray(normalizer, np.float32)[None, :], (B, NMEL))
    out = _PMAP(xf, mdiv.reshape(NCORES, ROWS, F), mbit.reshape(NCORES, ROWS, F),
                nrm.reshape(NCORES, ROWS, NMEL))
    return np.asarray(out).reshape(B, F, NMEL).astype(np.float32)


def _in_maps(x, T, normalizer):
    (xh_lo, xh_tl), (xl_lo, xl_tl), maskdiv, maskbit, nb = _host_prep(x, T, normalizer)
    in_maps = []
    for c in range(NCORES):
        r0 = c * ROWS
        sl = slice(r0, r0 + ROWS)
        in_maps.append({
            "xhlo": xh_lo[sl], "xhtl": xh_tl[sl],
            "xllo": xl_lo[sl], "xltl": xl_tl[sl],
            "ch": _CH, "cl": _CL, "bd": _BD, "nb": nb,
            "md": np.ascontiguousarray(maskdiv[sl].T),
            "mb": maskbit[sl].reshape(1, -1),
        })
    return in_maps


def kernel(x, T, normalizer):
    if int(os.environ.get("KERNEL_PMAP", "0")):
        return _jax_kernel(x, T, normalizer)
    global _NC
    if _NC is None:
        _NC = _build_program()
    in_maps = _in_maps(x, T, normalizer)
    trace = bool(int(os.environ.get("KERNEL_TRACE", "0")))
    res = run_bass_kernel_spmd(_NC, in_maps, list(range(NCORES)), trace=trace)
    if res.exec_time_ns is not None:
        print(f"HW exec time: {res.exec_time_ns} ns")
    return np.concatenate([res.results[c]["out"] for c in range(NCORES)], axis=0)



# revision 29
# speedup vs baseline: 1.1185x; 1.0520x over previous
"""FBANK kernel for Trainium2 (8 NeuronCores, pure data-parallel over batch).

Pipeline per core (8 batch rows): DFT-as-matmul in bf16 with selective
2x2-piece split precision for low fft bins (1..64), squares on ACT/DVE,
mel projection folded with the re^2+im^2 add via duplicated banks,
log+normalizer, and a tiny ragged masked-mean fixup via K=1/M=1 matmuls.
"""
import os
import numpy as np
import ml_dtypes

import concourse.bass as bass
import concourse.bacc as bacc
import concourse.tile as tile
from concourse import mybir
from concourse.bass_utils import run_bass_kernel_spmd

BF16 = ml_dtypes.bfloat16
F16 = np.float16

SR, WIN, SHIFT, NMEL, PRE = 16000.0, 400, 160, 80, 0.97
EPS = float(np.finfo(np.float64).eps)
B, L = 64, 240000
F = 1 + (L - WIN) // SHIFT          # 1498
ROWS = 8                            # batch rows per core
NCORES = 8
L_PAD = 240768                      # 1504*160 + slack for the +128 transpose
G = 1504                            # padded window-count per row
B0 = 64                             # fft bins 1..B0 get full split precision
NBINS = 199                         # fft bins 1..199 (bins 0, 200 have zero mel weight)

# C column layout: [cos1-64 | sin1-64 | cos65-192 | sin65-192 |
#                   cos193-199 | pad25 | sin193-199]  (sin block 32-aligned
#                   within M3 so the power-add partition bases are legal)
M_CHUNKS = 4       # M0 (precise, 128 cols), M1 (128), M2 (128), M3 (39)
M3_COLS = 39
NCOLS = 423
# k-chunks: (C row range, K, plane tile, g offset)
K_ROWS = [(0, 128), (160, 128), (320, 80), (128, 32), (288, 32)]
K_GOFF = [0, 1, 2, 0, 1]
K_TILE = ['lo', 'lo', 'lo', 'tl', 'tl']   # which X tile the rhs comes from

TILES = [(0, 512), (512, 512), (1024, 474)]   # (f0, NF) per row


def _build_consts():
    def hz2mel(f):
        return 1127.0 * np.log(1.0 + f / 700.0)
    mlow, mhigh = hz2mel(20.0), hz2mel(SR / 2.0)
    d = (mhigh - mlow) / (NMEL + 1)
    left = mlow + np.arange(NMEL) * d
    right = left + 2 * d
    fft_freqs = (SR / WIN) * np.arange(WIN // 2)
    mel = hz2mel(fft_freqs)[None, :]
    banks = np.maximum(0.0, np.minimum((mel - left[:, None]) / d,
                                       (right[:, None] - mel) / d))
    banks = np.concatenate([banks, np.zeros((NMEL, 1))], axis=1)  # (80, 201)

    w = 0.5 - 0.5 * np.cos(2 * np.pi * np.arange(WIN) / (WIN - 1))
    P = np.zeros((WIN, WIN))
    for t in range(WIN):
        P[t, t] += w[t]
        if t + 1 < WIN:
            P[t, t + 1] += -PRE * w[t + 1]
    P[0, 0] = (1 - PRE) * w[0]
    t_ = np.arange(WIN)
    k_ = np.arange(WIN // 2 + 1)
    ang = -2 * np.pi * np.outer(t_, k_) / WIN
    Cc = P @ np.cos(ang)          # (400, 201)
    Cs = P @ np.sin(ang)

    # reordered column layout (400, 423)
    C = np.concatenate([Cc[:, 1:B0 + 1], Cs[:, 1:B0 + 1],
                        Cc[:, B0 + 1:193], Cs[:, B0 + 1:193],
                        Cc[:, 193:200], np.zeros((WIN, 25)),
                        Cs[:, 193:200]], axis=1)
    C_h = C.astype(BF16)
    C_l = (C - C_h.astype(np.float64)).astype(BF16)

    # k-chunk stationary tiles (5, 128, NCOLS); rows beyond K zero
    ch = np.zeros((5, 128, NCOLS), BF16)
    cl = np.zeros((5, 128, 128), BF16)
    for i, (t0, K) in enumerate(K_ROWS):
        ch[i, :K, :] = C_h[t0:t0 + K, :]
        cl[i, :K, :] = C_l[t0:t0 + K, :128]

    bT = banks.T.astype(F16)      # (201, 80)
    # mel chunks over the squared DFT cols (duplicated banks fold re^2+im^2):
    # bd0 = [bins1-64 | bins1-64], bd1 = bins 65-192, bd2 = M3 39-row layout
    bd = np.zeros((3, 128, NMEL), F16)
    bd[0, 0:64] = bT[1:65];  bd[0, 64:128] = bT[1:65]
    bd[1, :] = bT[65:193]
    bd[2, 0:7] = bT[193:200]; bd[2, 32:39] = bT[193:200]
    return ch, cl, bd


_CH, _CL, _BD = _build_consts()
_NC = None


def _build_program():
    # Bacc (not Bass): its finalize() runs the sync-legalization passes
    # (move_matmul_waits_to_ldweights, generate_event_semaphores) that split
    # multi-wait instructions neuronxcc codegen rejects.
    nc = bacc.Bacc("TRN2", target_bir_lowering=False, debug=False)
    dt = mybir.dt
    xhlo = nc.dram_tensor("xhlo", [ROWS, 128, G], dt.bfloat16, kind="ExternalInput")
    xhtl = nc.dram_tensor("xhtl", [ROWS, 32, G], dt.bfloat16, kind="ExternalInput")
    xllo = nc.dram_tensor("xllo", [ROWS, 128, G], dt.bfloat16, kind="ExternalInput")
    xltl = nc.dram_tensor("xltl", [ROWS, 32, G], dt.bfloat16, kind="ExternalInput")
    ch = nc.dram_tensor("ch", [5, 128, NCOLS], dt.bfloat16, kind="ExternalInput")
    cl = nc.dram_tensor("cl", [5, 128, 128], dt.bfloat16, kind="ExternalInput")
    bd = nc.dram_tensor("bd", [3, 128, NMEL], dt.float16, kind="ExternalInput")
    nb = nc.dram_tensor("nb", [128, 12, NMEL], dt.float32, kind="ExternalInput")
    md = nc.dram_tensor("md", [128, ROWS], dt.float32, kind="ExternalInput")
    mb = nc.dram_tensor("mb", [1, ROWS * 128], dt.float32, kind="ExternalInput")
    out = nc.dram_tensor("out", [ROWS, F, NMEL], dt.float32, kind="ExternalOutput")

    from contextlib import ExitStack
    with tile.TileContext(nc) as tc, ExitStack() as ctx:
        singles = ctx.enter_context(tc.tile_pool(name="singles", bufs=1))
        xpool = ctx.enter_context(tc.tile_pool(name="xpool", bufs=8))
        sqpool = ctx.enter_context(tc.tile_pool(name="sqpool", bufs=6))
        fpool = ctx.enter_context(tc.tile_pool(name="fpool", bufs=3))
        dftps = ctx.enter_context(tc.tile_pool(name="dftps", bufs=1, space="PSUM"))
        melps = ctx.enter_context(tc.tile_pool(name="melps", bufs=2, space="PSUM"))
        fixps = ctx.enter_context(tc.tile_pool(name="fixps", bufs=1, space="PSUM"))

        ch_t = []
        cl_t = []
        for i in range(5):
            t = singles.tile([128, NCOLS], dt.bfloat16, tag=f"ch{i}")
            nc.sync.dma_start(t[:], ch[i])
            ch_t.append(t)
            t2 = singles.tile([128, 128], dt.bfloat16, tag=f"cl{i}")
            nc.sync.dma_start(t2[:], cl[i])
            cl_t.append(t2)
        bd_t = []
        for i in range(3):
            t = singles.tile([128, NMEL], dt.float16, tag=f"bd{i}")
            nc.sync.dma_start(t[:], bd[i])
            bd_t.append(t)
        nb_t = singles.tile([128, 12, NMEL], dt.float32, tag="nb")
        nc.sync.dma_start(nb_t[:], nb[:])
        md_t = singles.tile([128, ROWS], dt.float32, tag="md")
        nc.sync.dma_start(md_t[:], md[:])
        mb_t = singles.tile([1, ROWS * 128], dt.float32, tag="mb")
        nc.sync.dma_start(mb_t[:], mb[:])
        zeros_t = singles.tile([128, 1], dt.float32, tag="z")
        nc.vector.memset(zeros_t[:], 0.0)

        for r in range(ROWS):
            # --- load sample-major X planes (host pre-transposed); tl planes
            # only carry the 32 rows the K-chunks actually read ---
            xt = {}
            for pname, xsrc_lo, xsrc_tl in (("h", xhlo, xhtl), ("l", xllo, xltl)):
                xlo = xpool.tile([128, G], dt.bfloat16, tag=f"xlo{pname}")
                xtl = xpool.tile([32, G], dt.bfloat16, tag=f"xtl{pname}")
                nc.sync.dma_start(out=xlo[:], in_=xsrc_lo[r])
                nc.sync.dma_start(out=xtl[:], in_=xsrc_tl[r])
                xt[pname] = (xlo, xtl)

            stage = fpool.tile([128, 12, NMEL], dt.float32, tag="stage")
            nc.vector.memset(stage[64:128, 11, :], 1.0)
            for (f0, NF) in TILES:
                pA = dftps.tile([128, 2, 512], dt.float32, tag="pA")
                pB = dftps.tile([128, 2, 512], dt.float32, tag="pB")
                regions = [pA[:, 0, 0:NF], pA[:, 1, 0:NF],
                           pB[:, 0, 0:NF], pB[0:M3_COLS, 1, 0:NF]]
                mcols = [(0, 128), (128, 128), (256, 128), (384, M3_COLS)]

                def rhs_ap(plane, ki):
                    xlo, xtl = xt[plane]
                    t = xlo if K_TILE[ki] == 'lo' else xtl
                    K = K_ROWS[ki][1]
                    g = f0 + K_GOFF[ki]
                    return t[0:K, g:g + NF]

                # M0: 15 accumulating MMs (main + 2 cross passes)
                m0lo, m0n = mcols[0]
                for idx in range(15):
                    p, ki = idx // 5, idx % 5
                    K = K_ROWS[ki][1]
                    if p == 0:
                        lhs = ch_t[ki][0:K, m0lo:m0lo + m0n]
                        rhs = rhs_ap('h', ki)
                    elif p == 1:
                        lhs = cl_t[ki][0:K, 0:m0n]
                        rhs = rhs_ap('h', ki)
                    else:
                        lhs = ch_t[ki][0:K, m0lo:m0lo + m0n]
                        rhs = rhs_ap('l', ki)
                    nc.tensor.matmul(regions[0], lhs, rhs,
                                     start=(idx == 0), stop=(idx == 14))
                # M1..M3: 5 MMs each
                for mi in range(1, 4):
                    lo, n = mcols[mi]
                    for ki in range(5):
                        K = K_ROWS[ki][1]
                        nc.tensor.matmul(regions[mi],
                                         ch_t[ki][0:K, lo:lo + n],
                                         rhs_ap('h', ki),
                                         start=(ki == 0), stop=(ki == 4))

                # --- squares (psum fp32 -> sbuf fp16) on DVE; ACT was the
                # bottleneck engine for these wide elementwise ops ---
                sqA = sqpool.tile([128, 2, 512], dt.float16, tag="sqA")
                sqB = sqpool.tile([128, 2, 512], dt.float16, tag="sqB")
                cB = sqpool.tile([128, 2, 512], dt.float16, tag="cB")
                # Squares: DVE cannot read two PSUM operands, so the choice is
                # ACT Square (unary) or DVE copy-to-SBUF + mul. Alternate the
                # big regions between the two engines per tile to balance load.
                srcs = [(sqA, pA, 0, 128), (sqA, pA, 1, 128),
                        (sqB, pB, 0, 128), (sqB, pB, 1, M3_COLS)]
                on_act = (f0 // 512) % 2 == 0
                for si, (sq_t, p_t, pl, nr) in enumerate(srcs):
                    if on_act != (si % 2 == 0):
                        nc.scalar.activation(sq_t[0:nr, pl, 0:NF], p_t[0:nr, pl, 0:NF],
                                             mybir.ActivationFunctionType.Square,
                                             bias=zeros_t[0:nr])
                    else:
                        nc.vector.tensor_copy(cB[0:nr, pl, 0:NF], p_t[0:nr, pl, 0:NF])
                        nc.vector.tensor_mul(sq_t[0:nr, pl, 0:NF], cB[0:nr, pl, 0:NF],
                                             cB[0:nr, pl, 0:NF])

                # --- power pre-add for bins 65-192 (both operands base 0,
                # so the tensor_tensor SBUF base-partition rule is satisfied) ---
                pw = sqpool.tile([128, 512], dt.float16, tag="pw")
                nc.gpsimd.tensor_add(pw[:, 0:NF], sqA[:, 1, 0:NF], sqB[:, 0, 0:NF])

                # --- mel matmuls: out[frame, mel], 3 K-chunks ---
                mps = melps.tile([128, 4, NMEL], dt.float32, tag="mps")
                nsub = (NF + 127) // 128
                for j in range(nsub):
                    nj = min(128, NF - j * 128)
                    fr = slice(j * 128, j * 128 + nj)
                    chunks = [(sqA[:, 0, fr], 128, bd_t[0]),
                              (pw[:, fr], 128, bd_t[1]),
                              (sqB[0:M3_COLS, 1, fr], M3_COLS, bd_t[2])]
                    for ci, (lhs, K, bdt) in enumerate(chunks):
                        nc.tensor.matmul(mps[0:nj, j, :], lhs, bdt[0:K, :],
                                         start=(ci == 0), stop=(ci == 2))

                # --- clamp into per-row staging (Ln/mul happen once per row) ---
                ti = f0 // 512
                for j in range(nsub):
                    nj = min(128, NF - j * 128)
                    nc.vector.tensor_scalar_max(stage[0:nj, ti * 4 + j, :],
                                                mps[0:nj, j, :], EPS)

            # --- row tail: one Ln + one normalizer mul over all 12 subtiles ---
            out_st = fpool.tile([128, 12, NMEL], dt.float32, tag="out_st")
            nc.scalar.activation(out_st[:], stage[:],
                                 mybir.ActivationFunctionType.Ln,
                                 bias=zeros_t[:])
            nc.vector.tensor_mul(out_st[:], out_st[:], nb_t[:])

            # --- ragged masked-mean fixup (frames 0..127 = subtile 0) ---
            mean_ps = fixps.tile([1, NMEL], dt.float32, tag="mean")
            nc.tensor.matmul(mean_ps[:], md_t[:, r:r + 1], out_st[:, 0, :],
                             start=True, stop=True)
            mean_sb = fpool.tile([1, NMEL], dt.float32, tag="mean_sb")
            nc.vector.tensor_copy(mean_sb[:], mean_ps[:])
            fix_ps = fixps.tile([128, NMEL], dt.float32, tag="fix")
            nc.tensor.matmul(fix_ps[:], mb_t[0:1, r * 128:(r + 1) * 128],
                             mean_sb[:], start=True, stop=True)
            nc.vector.tensor_sub(out_st[:, 0, :], out_st[:, 0, :], fix_ps[:])

            # --- stores ---
            for js in range(12):
                nj = min(128, F - js * 128)
                nc.sync.dma_start(out[r, js * 128: js * 128 + nj, :],
                                  out_st[0:nj, js, :])
    nc.finalize()
    return nc


def _sample_major(plane):
    # plane (B, L_PAD) -> lo (B, 128, G): [b, v, g] = plane[b, 160g + v]
    #                     tl (B, 32, G):  [b, v, g] = plane[b, 160g + 128 + v]
    s = plane.strides
    lo = np.lib.stride_tricks.as_strided(
        plane, shape=(B, 128, G), strides=(s[0], s[1], 160 * s[1]))
    tl = np.lib.stride_tricks.as_strided(
        plane[:, 128:], shape=(B, 32, G), strides=(s[0], s[1], 160 * s[1]))
    return np.ascontiguousarray(lo), np.ascontiguousarray(tl)


_SCRATCH = None


def _host_prep(x, T, normalizer):
    global _SCRATCH
    xf = np.asarray(x, np.float32)
    if _SCRATCH is None:
        # pad region beyond L stays zero across calls; only [:, :L] is rewritten
        _SCRATCH = (np.zeros((B, L_PAD), BF16), np.zeros((B, L_PAD), BF16))
    xh0, xl0 = _SCRATCH
    xh0[:, :L] = xf.astype(BF16)
    xl0[:, :L] = (xf - xh0[:, :L].astype(np.float32)).astype(BF16)
    xh = _sample_major(xh0)
    xlo = _sample_major(xl0)

    T = np.asarray(T, np.int32)
    ds = T.max().astype(np.float32) / np.float32(NMEL)
    T_ = (T.astype(np.float32) / ds).astype(np.int32)
    cnt = np.maximum(T_, 1).astype(np.float32)
    f = np.arange(128)[None, :]
    maskbit = (f < T_[:, None]).astype(np.float32)          # (64, 128)
    maskdiv = maskbit / cnt[:, None]

    nrm = np.asarray(normalizer, np.float32)
    nb = np.broadcast_to(nrm[None, None, :], (128, 12, NMEL)).copy()
    return xh, xlo, maskdiv, maskbit, nb


_PMAP = None


def _jax_kernel(x, T, normalizer):
    """Data-parallel FBANK on the 8 NeuronCores via jax pmap (conv-based
    framing+DFT; no gather)."""
    global _PMAP
    import jax
    import jax.numpy as jnp

    def hz2mel(f):
        return 1127.0 * np.log(1.0 + f / 700.0)
    mlow, mhigh = hz2mel(20.0), hz2mel(SR / 2.0)
    d = (mhigh - mlow) / (NMEL + 1)
    left = mlow + np.arange(NMEL) * d
    fft_freqs = (SR / WIN) * np.arange(WIN // 2)
    melf = hz2mel(fft_freqs)[None, :]
    banks = np.maximum(0.0, np.minimum((melf - left[:, None]) / d,
                                       ((left[:, None] + 2 * d) - melf) / d))
    banksT = banks.T[1:200].astype(np.float32)           # (199, 80)

    w = 0.5 - 0.5 * np.cos(2 * np.pi * np.arange(WIN) / (WIN - 1))
    P = np.zeros((WIN, WIN))
    for t in range(WIN):
        P[t, t] += w[t]
        if t + 1 < WIN:
            P[t, t + 1] += -PRE * w[t + 1]
    P[0, 0] = (1 - PRE) * w[0]
    ang = -2 * np.pi * np.outer(np.arange(WIN), np.arange(1, 200)) / WIN
    Cc = (P @ np.cos(ang)).astype(np.float32)            # (400, 199)
    Cs = (P @ np.sin(ang)).astype(np.float32)
    filt = np.concatenate([Cc, Cs], axis=1).T[:, None, :]  # (398, 1, 400)

    if _PMAP is None:
        filt_j = jnp.asarray(filt)
        banksT_j = jnp.asarray(banksT)

        def shard_fn(xs, mdiv, mbit, nrm):
            spec = jax.lax.conv_general_dilated(
                xs[:, None, :], filt_j, window_strides=(SHIFT,),
                padding="VALID")                          # (r, 398, F)
            re, im = spec[:, :199, :], spec[:, 199:, :]
            power = re * re + im * im                     # (r, 199, F)
            mel = jnp.einsum("bkf,km->bfm", power, banksT_j)
            fb = jnp.log(jnp.maximum(mel, EPS)) * nrm[:, None, :]
            mean = jnp.einsum("bfm,bf->bm", fb, mdiv)[:, None, :]
            return fb - mean * mbit[:, :, None]

        _PMAP = jax.pmap(shard_fn)

    xf = np.asarray(x, np.float32).reshape(NCORES, ROWS, L)
    T = np.asarray(T, np.int32)
    ds = T.max().astype(np.float32) / np.float32(NMEL)
    T_ = (T.astype(np.float32) / ds).astype(np.int32)
    cnt = np.maximum(T_, 1).astype(np.float32)
    f = np.arange(F)[None, :]
    mbit = (f < T_[:, None]).astype(np.float32)           # (64, F)
    mdiv = mbit / cnt[:, None]
    nrm = np.broadcast_to(np.asarray(normalizer, np.float32)[None, :], (B, NMEL))
    out = _PMAP(xf, mdiv.reshape(NCORES, ROWS, F), mbit.reshape(NCORES, ROWS, F),
                nrm.reshape(NCORES, ROWS, NMEL))
    return np.asarray(out).reshape(B, F, NMEL).astype(np.float32)


def _in_maps(x, T, normalizer):
    (xh_lo, xh_tl), (xl_lo, xl_tl), maskdiv, maskbit, nb = _host_prep(x, T, normalizer)
    in_maps = []
    for c in range(NCORES):
        r0 = c * ROWS
        sl = slice(r0, r0 + ROWS)
        in_maps.append({
            "xhlo": xh_lo[sl], "xhtl": xh_tl[sl],
            "xllo": xl_lo[sl], "xltl": xl_tl[sl],
            "ch": _CH, "cl": _CL, "bd": _BD, "nb": nb,
            "md": np.ascontiguousarray(maskdiv[sl].T),
            "mb": maskbit[sl].reshape(1, -1),
        })
    return in_maps


def kernel(x, T, normalizer):
    if int(os.environ.get("KERNEL_PMAP", "0")):
        return _jax_kernel(x, T, normalizer)
    global _NC
    if _NC is None:
        _NC = _build_program()
    in_maps = _in_maps(x, T, normalizer)
    trace = bool(int(os.environ.get("KERNEL_TRACE", "0")))
    res = run_bass_kernel_spmd(_NC, in_maps, list(range(NCORES)), trace=trace)
    if res.exec_time_ns is not None:
        print(f"HW exec time: {res.exec_time_ns} ns")
    return np.concatenate([res.results[c]["out"] for c in range(NCORES)], axis=0)

